# revision 47
# baseline (speedup 1.0000x reference)
"""Trainium2 Bass kernel for LISTA (nn_LISTA_37976100831401).

Data-parallel sharding: batch 16384 -> 8 NeuronCores x 2048 rows.
We / S / theta are replicated on every core; no cross-device communication.

Per-core algorithm:
  B  = X @ We.T                 (2048, 1024)
  Z0 = soft(B);  Z_{t+1} = soft(B + Z_t @ S.T), t = 0..15
  soft(x) = relu(x - theta) - relu(-x - theta)

All matmul operands are bf16 (PSUM accumulation stays fp32).  On TRN2 the
measured back-to-back N=512 matmul period is 216 ns for bf16 vs 227 ns for
fp32r (the fp32r weight path costs ~13 ns/matmul extra), so bf16 runs the
2048x1024x1024 step matmuls at the PE streaming roofline.  Numerically the
bf16 pipeline lands at ~7e-3 relative error (gate is 2e-2): quantization
noise (~0.4%/step) does not compound destructively through the 16
soft-threshold steps.

Everything stays in the [feature, batch] orientation for all 16 steps:
  C.T = S @ Z.T + B.T  via  psum[j,b] += ST[k][:, j128].T @ ZT[k][:, b512]
so the matmul OUTPUT layout equals the INPUT layout of the next step.  The
device writes Z.T ([1024, 2048] per core) and the host transposes while
gathering -- this removes the baseline's flipped final step that re-derived
a batch-major layout by re-accumulating X@We.T on the PE.

The four 512-column batch chunks advance round-robin (c0..c3 per step), so
while chunk c's PSUM groups drain through DVE/ACT the PE streams the other
three chunks' matmuls (~41 us of cover for a ~3 us chain).

Opening: ~24 throwaway bf16 matmuls on memset tiles start as soon as the PE
engine boots, warming the HAM clock gate (1.2 -> 2.4 GHz takes ~3.4 us of
sustained PE activity) while the input DMAs run; S.T streams on the scalar
ring concurrently with We/X on the sync ring so the first step round never
waits on S.
"""

import time
from contextlib import ExitStack

import numpy as np
import ml_dtypes

import concourse.bacc as bacc
import concourse.mybir as mybir
import concourse.tile as tile
from concourse import bass_utils

FP32 = mybir.dt.float32
BF16 = mybir.dt.bfloat16
AL = mybir.AluOpType
AF = mybir.ActivationFunctionType

N_CORES = 8
B_TOTAL, N_IN, M = 16384, 256, 1024
B_CORE = B_TOTAL // N_CORES  # 2048
T_STEPS = 16                 # scan length in the reference
CHUNK = 512                  # batch columns per PSUM group (= bank / max N)
N_CHUNKS = B_CORE // CHUNK   # 4
KT = M // 128                # 8 feature tiles of 128
NT = N_IN // 128             # 2 input-feature tiles
N_WARM = 26                  # HAM warmup matmuls
WARM_N = 256                 # moving width of warmup matmuls


def _emit(ctx: ExitStack, tc: tile.TileContext, XT, WeT, ST, THP, ZT):
    nc = tc.nc

    const_pool = ctx.enter_context(tc.tile_pool(name="const", bufs=1))
    psum_pool = ctx.enter_context(tc.tile_pool(name="psum", bufs=1, space="PSUM"))
    xt_pool = ctx.enter_context(tc.tile_pool(name="xt", bufs=1))
    bt_pool = ctx.enter_context(tc.tile_pool(name="bt", bufs=1))
    zt_pool = ctx.enter_context(tc.tile_pool(name="zt", bufs=1))
    tmp_pool = ctx.enter_context(tc.tile_pool(name="tmp", bufs=1))
    out_pool = ctx.enter_context(tc.tile_pool(name="zout", bufs=1))

    # ---- constants -------------------------------------------------------
    warm_a = const_pool.tile([128, 128], BF16, name="warm_a")
    warm_m = const_pool.tile([128, WARM_N], BF16, name="warm_m")
    # theta ships host-marshaled in one padded [128, 128] fp32 tile (512B
    # DMA lines; a bare [128, 8] layout would be 32B lines = 128 tiny
    # descriptors that stall the whole ring): cols 0..KT-1 hold
    # th[p, kt] = theta[kt*128+p], cols KT..2KT-1 hold -theta.
    thpad = const_pool.tile([128, 128], FP32, name="thpad")

    def th_col(jt):
        return thpad[:, jt : jt + 1]

    def nth_col(jt):
        return thpad[:, KT + jt : KT + jt + 1]
    wet = [const_pool.tile([128, M], BF16, name=f"wet{nt}") for nt in range(NT)]
    st = [const_pool.tile([128, M], BF16, name=f"st{kt}") for kt in range(KT)]

    def emit_warmup():
        # PE work with zero DMA dependencies: warms the HAM clock gate while
        # inputs stream in.  Results are never read.
        nc.vector.memset(warm_a[:], 1.0)
        nc.vector.memset(warm_m[:], 0.5)
        for i in range(N_WARM):
            pw = psum_pool.tile([128, WARM_N], FP32, name="pw", tag="tp", bufs=2)
            nc.tensor.matmul(pw[:], warm_a[:], warm_m[:], start=True, stop=True)

    def emit_dma_in():
        # Three DMA rings, ordered so the first b-group's inputs (We + X
        # chunk 0) land first on ALL rings, then S.T streams behind (it is
        # only needed by the first step round, ~25us in):
        #   scalar: theta layouts, We half 0, S.T tiles 0-3
        #   sync:   We half 1, X.T chunks 1-3
        #   gpsimd: X.T chunk 0, S.T tiles 4-7
        # Ring speeds vary run-to-run (the gpsimd-triggered ring is the
        # slowest), so the first b-group's four operand tiles lead three
        # different rings; S.T streams behind (needed only by the first
        # step round, ~25us in), with the slow gpsimd ring carrying the
        # mid-order tiles.
        nc.gpsimd.dma_start(thpad[:], THP)
        nc.scalar.dma_start(wet[0][:], WeT[0:128, :])
        nc.sync.dma_start(wet[1][:], WeT[128:256, :])
        xts[0] = [
            xt_pool.tile([128, CHUNK], BF16, name=f"xt{nt}", tag=f"xt{nt}", bufs=4)
            for nt in range(NT)
        ]
        nc.sync.dma_start(xts[0][0][:], XT[0:128, 0:CHUNK])
        nc.gpsimd.dma_start(xts[0][1][:], XT[128:256, 0:CHUNK])
        for c in range(1, N_CHUNKS):
            x_phase(c, nc.sync)
        st_eng = {0: nc.scalar, 1: nc.scalar, 2: nc.scalar, 3: nc.sync,
                  4: nc.gpsimd, 5: nc.gpsimd, 6: nc.gpsimd, 7: nc.sync}
        for kt in range(KT):
            st_eng[kt].dma_start(st[kt][:], ST[kt * 128 : (kt + 1) * 128, :])

    xts = {}  # chunk -> [NT] tiles [128, CHUNK]

    def x_phase(c, dma_eng):
        xts[c] = [
            xt_pool.tile([128, CHUNK], BF16, name=f"xt{nt}", tag=f"xt{nt}", bufs=4)
            for nt in range(NT)
        ]
        for nt in range(NT):
            dma_eng.dma_start(
                xts[c][nt][:],
                XT[nt * 128 : (nt + 1) * 128, c * CHUNK : (c + 1) * CHUNK],
            )

    # ---- per-chunk state -------------------------------------------------
    bts = {}  # chunk -> [KT] tiles [128, CHUNK] bf16  (B.T slab)
    zts = {}  # chunk -> [KT] tiles [128, CHUNK] bf16  (current Z.T)

    def b_group(c, jt):
        # One tile of B.T = We @ X.T ; Z0 = soft(B).  Each b-group is only
        # ~432 ns of PE work (K=256), so PSUM bank turnaround gates the PE:
        # the bank is read exactly once by two parallel half-copies
        # (ACT + DVE, ~450 ns release), and Z0 derives from the bf16 B tile:
        #   soft(B) = relu(B - th) + min(B + th, 0)
        # Alternate between the two PSUM rings so b-groups see all 8 banks.
        tag, nbufs = ("mm", 6) if jt % 2 == 0 else ("tp", 2)
        ps = psum_pool.tile([128, CHUNK], FP32, name="psb", tag=tag, bufs=nbufs)
        for nt in range(NT):
            nc.tensor.matmul(
                ps[:],
                wet[nt][:, jt * 128 : (jt + 1) * 128],
                xts[c][nt][:],
                start=(nt == 0),
                stop=(nt == NT - 1),
            )
        btile = bt_pool.tile(
            [128, CHUNK], BF16, name="btile", tag=f"bt{jt}", bufs=4
        )
        half = CHUNK // 2
        nc.scalar.copy(btile[:, :half], ps[:, :half])
        nc.vector.tensor_copy(btile[:, half:], ps[:, half:])
        af = tmp_pool.tile([128, CHUNK], BF16, name="afb", tag="af", bufs=3)
        nc.scalar.activation(
            af[:], btile[:], AF.Relu, bias=nth_col(jt), scale=1.0
        )
        df = tmp_pool.tile([128, CHUNK], BF16, name="dfb", tag="df", bufs=3)
        nc.vector.tensor_scalar(
            df[:], btile[:], th_col(jt), 0.0, op0=AL.add, op1=AL.min
        )
        z0 = zt_pool.tile([128, CHUNK], BF16, name="z0", tag=f"zt{jt}", bufs=5)
        nc.vector.tensor_add(z0[:], af[:], df[:])
        bts[c].append(btile)
        zts[c].append(z0)

    def b_phase(c):
        bts[c] = []
        zts[c] = []
        for jt in range(KT):
            b_group(c, jt)

    def step_group(c, jt, zcur, znew):
        # One tile of Z <- soft(B + Z @ S.T), in the [j, b] orientation.
        ps = psum_pool.tile([128, CHUNK], FP32, name="pss", tag="mm", bufs=6)
        for kt in range(KT):
            nc.tensor.matmul(
                ps[:],
                st[kt][:, jt * 128 : (jt + 1) * 128],
                zcur[kt][:],
                start=(kt == 0),
                stop=(kt == KT - 1),
            )
        ct = tmp_pool.tile([128, CHUNK], BF16, name="ct", tag="ct", bufs=3)
        nc.vector.tensor_add(ct[:], ps[:], bts[c][jt][:])
        af = tmp_pool.tile([128, CHUNK], BF16, name="afs", tag="af", bufs=3)
        nc.scalar.activation(
            af[:], ct[:], AF.Relu, bias=nth_col(jt), scale=1.0
        )
        df = tmp_pool.tile([128, CHUNK], BF16, name="dfs", tag="df", bufs=3)
        nc.scalar.activation(
            df[:], ct[:], AF.Relu, bias=nth_col(jt), scale=-1.0
        )
        zn = zt_pool.tile([128, CHUNK], BF16, name="zn", tag=f"zt{jt}", bufs=5)
        nc.vector.tensor_sub(zn[:], af[:], df[:])
        znew.append(zn)

    def step(c, weave_b=None):
        # weave_b: chunk whose b-groups are interleaved 1:1 with this step's
        # groups, so each b-PSUM-bank gets ~2.2 us of PE cover to drain.
        zcur = zts[c]
        znew = []
        for jt in range(KT):
            step_group(c, jt, zcur, znew)
            if weave_b is not None:
                b_group(weave_b, jt)
        zts[c] = znew

    def final_step(c, last=False):
        # Last step keeps fp32 all the way to the output tile; Z.T DMAs out
        # row-contiguous (the host transposes while gathering).
        zcur = zts[c]
        for jt in range(KT):
            ps = psum_pool.tile([128, CHUNK], FP32, name="psf", tag="mm", bufs=6)
            for kt in range(KT):
                nc.tensor.matmul(
                    ps[:],
                    st[kt][:, jt * 128 : (jt + 1) * 128],
                    zcur[kt][:],
                    start=(kt == 0),
                    stop=(kt == KT - 1),
                )
            if last and jt == KT - 1:
                # Very last tile: halve the op width and run the relu (ACT)
                # against the min-branch (DVE) in parallel to shorten the
                # post-matmul critical chain that sets the kernel tail.
                #   soft(C) = relu(C - th) + min(C + th, 0)
                half = CHUNK // 2
                cf = tmp_pool.tile([128, CHUNK], FP32, name="cf", tag="cf", bufs=3)
                af = tmp_pool.tile([128, CHUNK], FP32, name="aff", tag="af", bufs=3)
                df = tmp_pool.tile([128, CHUNK], FP32, name="dff", tag="df", bufs=3)
                zo = out_pool.tile([128, CHUNK], FP32, name="zo", tag="zo", bufs=4)
                for h in (slice(0, half), slice(half, CHUNK)):
                    nc.vector.tensor_add(cf[:, h], ps[:, h], bts[c][jt][:, h])
                    nc.scalar.activation(
                        af[:, h], cf[:, h], AF.Relu,
                        bias=nth_col(jt), scale=1.0,
                    )
                    nc.vector.tensor_scalar(
                        df[:, h], cf[:, h], th_col(jt), 0.0,
                        op0=AL.add, op1=AL.min,
                    )
                    nc.vector.tensor_add(zo[:, h], af[:, h], df[:, h])
                    dma_eng = nc.sync if h.start == 0 else nc.scalar
                    dma_eng.dma_start(
                        ZT[jt * 128 : (jt + 1) * 128,
                           c * CHUNK + h.start : c * CHUNK + h.stop],
                        zo[:, h],
                    )
                continue
            cf = tmp_pool.tile([128, CHUNK], FP32, name="cf", tag="cf", bufs=3)
            nc.vector.tensor_add(cf[:], ps[:], bts[c][jt][:])
            af = tmp_pool.tile([128, CHUNK], FP32, name="aff", tag="af", bufs=3)
            nc.scalar.activation(
                af[:], cf[:], AF.Relu, bias=nth_col(jt), scale=1.0
            )
            df = tmp_pool.tile([128, CHUNK], FP32, name="dff", tag="df", bufs=3)
            nc.scalar.activation(
                df[:], cf[:], AF.Relu, bias=nth_col(jt), scale=-1.0
            )
            zo = out_pool.tile([128, CHUNK], FP32, name="zo", tag="zo", bufs=4)
            nc.vector.tensor_sub(zo[:], af[:], df[:])
            dma_eng = nc.sync if jt % 2 == 0 else nc.scalar
            dma_eng.dma_start(
                ZT[jt * 128 : (jt + 1) * 128, c * CHUNK : (c + 1) * CHUNK], zo[:]
            )

    # ---- emission schedule ----------------------------------------------
    emit_warmup()
    emit_dma_in()
    b_phase(0)
    b_phase(1)
    bts[2] = []
    zts[2] = []
    bts[3] = []
    zts[3] = []
    step(0, weave_b=2)
    step(1, weave_b=3)
    step(2)
    step(3)
    for _ in range(T_STEPS - 2):
        for c in range(N_CHUNKS):
            step(c)
    for c in range(N_CHUNKS):
        final_step(c, last=(c == N_CHUNKS - 1))


def build_nc():
    nc = bacc.Bacc("TRN2", target_bir_lowering=False, debug=False)
    XT = nc.dram_tensor("XT", [N_IN, B_CORE], BF16, kind="ExternalInput")
    WeT = nc.dram_tensor("WeT", [N_IN, M], BF16, kind="ExternalInput")
    ST = nc.dram_tensor("ST", [M, M], BF16, kind="ExternalInput")
    THP = nc.dram_tensor("THP", [128, 128], FP32, kind="ExternalInput")
    ZT = nc.dram_tensor("ZT", [M, B_CORE], FP32, kind="ExternalOutput")
    with tile.TileContext(nc) as tc:
        with ExitStack() as ctx:
            _emit(
                ctx, tc, XT.ap(), WeT.ap(), ST.ap(), THP.ap(), ZT.ap(),
            )
    nc.compile()
    return nc


_NC_CACHE = None


def _get_nc():
    global _NC_CACHE
    if _NC_CACHE is None:
        _NC_CACHE = build_nc()
    return _NC_CACHE


def make_in_maps(X, We, S, theta):
    X = np.asarray(X, dtype=np.float32)
    WeT = np.ascontiguousarray(np.asarray(We, dtype=np.float32).T).astype(
        ml_dtypes.bfloat16
    )
    ST = np.ascontiguousarray(np.asarray(S, dtype=np.float32).T).astype(
        ml_dtypes.bfloat16
    )
    theta = np.asarray(theta, dtype=np.float32)
    # Padded [128, 128] per-partition layout: col kt holds theta[kt*128+p],
    # col KT+kt holds -theta[kt*128+p]; remaining columns are zero.
    thp = np.zeros((128, 128), dtype=np.float32)
    thp[:, :KT] = theta.reshape(KT, 128).T
    thp[:, KT : 2 * KT] = -thp[:, :KT]
    return [
        {
            "XT": np.ascontiguousarray(X[i * B_CORE : (i + 1) * B_CORE].T).astype(
                ml_dtypes.bfloat16
            ),
            "WeT": WeT,
            "ST": ST,
            "THP": thp,
        }
        for i in range(N_CORES)
    ]


def gather_out(results):
    return np.concatenate(
        [
            np.asarray(results[i]["ZT"], dtype=np.float32).T
            for i in range(N_CORES)
        ],
        axis=0,
    )


def run(X, We, S, theta, trace=False, **trace_kwargs):
    nc = _get_nc()
    in_maps = make_in_maps(X, We, S, theta)
    # The PJRT compile callback can fail transiently ("CallFunctionObjArgs");
    # a retry in the same process succeeds.
    last_err = None
    for _attempt in range(3):
        try:
            res = bass_utils.run_bass_kernel_spmd(
                nc, in_maps, list(range(N_CORES)), trace=trace, **trace_kwargs
            )
            break
        except Exception as e:  # noqa: BLE001
            last_err = e
            time.sleep(2.0)
    else:
        raise last_err
    Z = gather_out(res.results)
    return Z.astype(np.float32, copy=False), res


def kernel(X, We, S, theta):
    Z, _ = run(X, We, S, theta, trace=False)
    return Z


# revision 48
# speedup vs baseline: 1.0014x; 1.0014x over previous
"""Trainium2 Bass kernel for LISTA (nn_LISTA_37976100831401).

Data-parallel sharding: batch 16384 -> 8 NeuronCores x 2048 rows.
We / S / theta are replicated on every core; no cross-device communication.

Per-core algorithm:
  B  = X @ We.T                 (2048, 1024)
  Z0 = soft(B);  Z_{t+1} = soft(B + Z_t @ S.T), t = 0..15
  soft(x) = relu(x - theta) - relu(-x - theta)

All matmul operands are bf16 (PSUM accumulation stays fp32).  On TRN2 the
measured back-to-back N=512 matmul period is 216 ns for bf16 vs 227 ns for
fp32r (the fp32r weight path costs ~13 ns/matmul extra), so bf16 runs the
2048x1024x1024 step matmuls at the PE streaming roofline.  Numerically the
bf16 pipeline lands at ~7e-3 relative error (gate is 2e-2): quantization
noise (~0.4%/step) does not compound destructively through the 16
soft-threshold steps.

Everything stays in the [feature, batch] orientation for all 16 steps:
  C.T = S @ Z.T + B.T  via  psum[j,b] += ST[k][:, j128].T @ ZT[k][:, b512]
so the matmul OUTPUT layout equals the INPUT layout of the next step.  The
device writes Z.T ([1024, 2048] per core) and the host transposes while
gathering -- this removes the baseline's flipped final step that re-derived
a batch-major layout by re-accumulating X@We.T on the PE.

The four 512-column batch chunks advance round-robin (c0..c3 per step), so
while chunk c's PSUM groups drain through DVE/ACT the PE streams the other
three chunks' matmuls (~41 us of cover for a ~3 us chain).

Opening: throwaway bf16 matmuls on memset tiles start as soon as the PE
engine boots, warming the HAM clock gate (1.2 -> 2.4 GHz takes ~3.4 us of
sustained PE activity) while the input DMAs run across all three DMA rings
(sync / scalar / gpsimd-triggered), so the b-phase and the first step round
start with their operands resident and the clock warm.
"""

import time
from contextlib import ExitStack

import numpy as np
import ml_dtypes

import concourse.bacc as bacc
import concourse.mybir as mybir
import concourse.tile as tile
from concourse import bass_utils

FP32 = mybir.dt.float32
BF16 = mybir.dt.bfloat16
AL = mybir.AluOpType
AF = mybir.ActivationFunctionType

N_CORES = 8
B_TOTAL, N_IN, M = 16384, 256, 1024
B_CORE = B_TOTAL // N_CORES  # 2048
T_STEPS = 16                 # scan length in the reference
CHUNK = 512                  # batch columns per PSUM group (= bank / max N)
N_CHUNKS = B_CORE // CHUNK   # 4
KT = M // 128                # 8 feature tiles of 128
NT = N_IN // 128             # 2 input-feature tiles
N_WARM = 18                  # HAM warmup matmuls
WARM_N = 256                 # moving width of warmup matmuls


def _emit(ctx: ExitStack, tc: tile.TileContext, XT, WeT, ST, THP, ZT):
    nc = tc.nc

    const_pool = ctx.enter_context(tc.tile_pool(name="const", bufs=1))
    psum_pool = ctx.enter_context(tc.tile_pool(name="psum", bufs=1, space="PSUM"))
    xt_pool = ctx.enter_context(tc.tile_pool(name="xt", bufs=1))
    bt_pool = ctx.enter_context(tc.tile_pool(name="bt", bufs=1))
    zt_pool = ctx.enter_context(tc.tile_pool(name="zt", bufs=1))
    tmp_pool = ctx.enter_context(tc.tile_pool(name="tmp", bufs=1))
    out_pool = ctx.enter_context(tc.tile_pool(name="zout", bufs=1))

    # ---- constants -------------------------------------------------------
    warm_a = const_pool.tile([128, 128], BF16, name="warm_a")
    warm_m = const_pool.tile([128, WARM_N], BF16, name="warm_m")
    # theta ships host-marshaled in one padded [128, 128] fp32 tile (512B
    # DMA lines; a bare [128, 8] layout would be 32B lines = 128 tiny
    # descriptors that stall the whole ring): cols 0..KT-1 hold
    # th[p, kt] = theta[kt*128+p], cols KT..2KT-1 hold -theta.
    thpad = const_pool.tile([128, 128], FP32, name="thpad")

    def th_col(jt):
        return thpad[:, jt : jt + 1]

    def nth_col(jt):
        return thpad[:, KT + jt : KT + jt + 1]
    wet = [const_pool.tile([128, M], BF16, name=f"wet{nt}") for nt in range(NT)]
    st = [const_pool.tile([128, M], BF16, name=f"st{kt}") for kt in range(KT)]

    def emit_warmup():
        # PE work with zero DMA dependencies: warms the HAM clock gate while
        # inputs stream in.  Results are never read.
        nc.vector.memset(warm_a[:], 1.0)
        nc.vector.memset(warm_m[:], 0.5)
        for i in range(N_WARM):
            pw = psum_pool.tile([128, WARM_N], FP32, name="pw", tag="tp", bufs=2)
            nc.tensor.matmul(pw[:], warm_a[:], warm_m[:], start=True, stop=True)

    def emit_dma_in():
        # Ring speeds vary run-to-run (the gpsimd-triggered ring is the
        # slowest), so the first b-group's four operand tiles lead three
        # different rings; S.T streams behind (needed only by the first
        # step round, ~25us in), with the slow gpsimd ring carrying the
        # mid-order tiles.
        nc.gpsimd.dma_start(thpad[:], THP)
        nc.scalar.dma_start(wet[0][:], WeT[0:128, :])
        nc.sync.dma_start(wet[1][:], WeT[128:256, :])
        xts[0] = [
            xt_pool.tile([128, CHUNK], BF16, name=f"xt{nt}", tag=f"xt{nt}", bufs=4)
            for nt in range(NT)
        ]
        nc.sync.dma_start(xts[0][0][:], XT[0:128, 0:CHUNK])
        nc.gpsimd.dma_start(xts[0][1][:], XT[128:256, 0:CHUNK])
        for c in range(1, N_CHUNKS):
            x_phase(c, nc.sync)
        st_eng = {0: nc.scalar, 1: nc.scalar, 2: nc.scalar, 3: nc.sync,
                  4: nc.gpsimd, 5: nc.gpsimd, 6: nc.gpsimd, 7: nc.sync}
        for kt in range(KT):
            st_eng[kt].dma_start(st[kt][:], ST[kt * 128 : (kt + 1) * 128, :])

    xts = {}  # chunk -> [NT] tiles [128, CHUNK]

    def x_phase(c, dma_eng):
        xts[c] = [
            xt_pool.tile([128, CHUNK], BF16, name=f"xt{nt}", tag=f"xt{nt}", bufs=4)
            for nt in range(NT)
        ]
        for nt in range(NT):
            dma_eng.dma_start(
                xts[c][nt][:],
                XT[nt * 128 : (nt + 1) * 128, c * CHUNK : (c + 1) * CHUNK],
            )

    # ---- per-chunk state -------------------------------------------------
    bts = {}  # chunk -> [KT] tiles [128, CHUNK] bf16  (B.T slab)
    zts = {}  # chunk -> [KT] tiles [128, CHUNK] bf16  (current Z.T)

    def b_group(c, jt):
        # One tile of B.T = We @ X.T ; Z0 = soft(B).  Each b-group is only
        # ~432 ns of PE work (K=256), so PSUM bank turnaround gates the PE:
        # the bank is read exactly once by two parallel half-copies
        # (ACT + DVE, ~450 ns release), and Z0 derives from the bf16 B tile:
        #   soft(B) = relu(B - th) + min(B + th, 0)
        # Alternate between the two PSUM rings so b-groups see all 8 banks.
        tag, nbufs = ("mm", 6) if jt % 2 == 0 else ("tp", 2)
        ps = psum_pool.tile([128, CHUNK], FP32, name="psb", tag=tag, bufs=nbufs)
        for nt in range(NT):
            nc.tensor.matmul(
                ps[:],
                wet[nt][:, jt * 128 : (jt + 1) * 128],
                xts[c][nt][:],
                start=(nt == 0),
                stop=(nt == NT - 1),
            )
        btile = bt_pool.tile(
            [128, CHUNK], BF16, name="btile", tag=f"bt{jt}", bufs=4
        )
        half = CHUNK // 2
        nc.scalar.copy(btile[:, :half], ps[:, :half])
        nc.vector.tensor_copy(btile[:, half:], ps[:, half:])
        af = tmp_pool.tile([128, CHUNK], BF16, name="afb", tag="af", bufs=3)
        nc.scalar.activation(
            af[:], btile[:], AF.Relu, bias=nth_col(jt), scale=1.0
        )
        df = tmp_pool.tile([128, CHUNK], BF16, name="dfb", tag="df", bufs=3)
        nc.vector.tensor_scalar(
            df[:], btile[:], th_col(jt), 0.0, op0=AL.add, op1=AL.min
        )
        z0 = zt_pool.tile([128, CHUNK], BF16, name="z0", tag=f"zt{jt}", bufs=5)
        nc.vector.tensor_add(z0[:], af[:], df[:])
        bts[c].append(btile)
        zts[c].append(z0)

    def b_phase(c):
        bts[c] = []
        zts[c] = []
        for jt in range(KT):
            b_group(c, jt)

    def step_group(c, jt, zcur, znew):
        # One tile of Z <- soft(B + Z @ S.T), in the [j, b] orientation.
        ps = psum_pool.tile([128, CHUNK], FP32, name="pss", tag="mm", bufs=6)
        for kt in range(KT):
            nc.tensor.matmul(
                ps[:],
                st[kt][:, jt * 128 : (jt + 1) * 128],
                zcur[kt][:],
                start=(kt == 0),
                stop=(kt == KT - 1),
            )
        ct = tmp_pool.tile([128, CHUNK], BF16, name="ct", tag="ct", bufs=3)
        nc.vector.tensor_add(ct[:], ps[:], bts[c][jt][:])
        af = tmp_pool.tile([128, CHUNK], BF16, name="afs", tag="af", bufs=3)
        nc.scalar.activation(
            af[:], ct[:], AF.Relu, bias=nth_col(jt), scale=1.0
        )
        df = tmp_pool.tile([128, CHUNK], BF16, name="dfs", tag="df", bufs=3)
        nc.scalar.activation(
            df[:], ct[:], AF.Relu, bias=nth_col(jt), scale=-1.0
        )
        zn = zt_pool.tile([128, CHUNK], BF16, name="zn", tag=f"zt{jt}", bufs=5)
        nc.vector.tensor_sub(zn[:], af[:], df[:])
        znew.append(zn)

    def step(c, weave_b=None):
        # weave_b: chunk whose b-groups are interleaved 1:1 with this step's
        # groups, so each b-PSUM-bank gets ~2.2 us of PE cover to drain.
        zcur = zts[c]
        znew = []
        for jt in range(KT):
            step_group(c, jt, zcur, znew)
            if weave_b is not None:
                b_group(weave_b, jt)
        zts[c] = znew

    def final_step(c, last=False):
        # Last step keeps fp32 all the way to the output tile; Z.T DMAs out
        # row-contiguous (the host transposes while gathering).
        zcur = zts[c]
        for jt in range(KT):
            ps = psum_pool.tile([128, CHUNK], FP32, name="psf", tag="mm", bufs=6)
            for kt in range(KT):
                nc.tensor.matmul(
                    ps[:],
                    st[kt][:, jt * 128 : (jt + 1) * 128],
                    zcur[kt][:],
                    start=(kt == 0),
                    stop=(kt == KT - 1),
                )
            if last and jt == KT - 1:
                # Very last tile: halve the op width and run the relu (ACT)
                # against the min-branch (DVE) in parallel to shorten the
                # post-matmul critical chain that sets the kernel tail.
                #   soft(C) = relu(C - th) + min(C + th, 0)
                half = CHUNK // 2
                cf = tmp_pool.tile([128, CHUNK], FP32, name="cf", tag="cf", bufs=3)
                af = tmp_pool.tile([128, CHUNK], FP32, name="aff", tag="af", bufs=3)
                df = tmp_pool.tile([128, CHUNK], FP32, name="dff", tag="df", bufs=3)
                zo = out_pool.tile([128, CHUNK], FP32, name="zo", tag="zo", bufs=4)
                for h in (slice(0, half), slice(half, CHUNK)):
                    nc.vector.tensor_add(cf[:, h], ps[:, h], bts[c][jt][:, h])
                    nc.scalar.activation(
                        af[:, h], cf[:, h], AF.Relu,
                        bias=nth_col(jt), scale=1.0,
                    )
                    nc.vector.tensor_scalar(
                        df[:, h], cf[:, h], th_col(jt), 0.0,
                        op0=AL.add, op1=AL.min,
                    )
                    nc.vector.tensor_add(zo[:, h], af[:, h], df[:, h])
                    dma_eng = nc.sync if h.start == 0 else nc.scalar
                    dma_eng.dma_start(
                        ZT[jt * 128 : (jt + 1) * 128,
                           c * CHUNK + h.start : c * CHUNK + h.stop],
                        zo[:, h],
                    )
                continue
            cf = tmp_pool.tile([128, CHUNK], FP32, name="cf", tag="cf", bufs=3)
            nc.vector.tensor_add(cf[:], ps[:], bts[c][jt][:])
            af = tmp_pool.tile([128, CHUNK], FP32, name="aff", tag="af", bufs=3)
            nc.scalar.activation(
                af[:], cf[:], AF.Relu, bias=nth_col(jt), scale=1.0
            )
            df = tmp_pool.tile([128, CHUNK], FP32, name="dff", tag="df", bufs=3)
            nc.scalar.activation(
                df[:], cf[:], AF.Relu, bias=nth_col(jt), scale=-1.0
            )
            zo = out_pool.tile([128, CHUNK], FP32, name="zo", tag="zo", bufs=4)
            nc.vector.tensor_sub(zo[:], af[:], df[:])
            dma_eng = nc.sync if jt % 2 == 0 else nc.scalar
            dma_eng.dma_start(
                ZT[jt * 128 : (jt + 1) * 128, c * CHUNK : (c + 1) * CHUNK], zo[:]
            )

    # ---- emission schedule ----------------------------------------------
    emit_warmup()
    emit_dma_in()
    b_phase(0)
    b_phase(1)
    bts[2] = []
    zts[2] = []
    bts[3] = []
    zts[3] = []
    step(0, weave_b=2)
    step(1, weave_b=3)
    step(2)
    step(3)
    for _ in range(T_STEPS - 2):
        for c in range(N_CHUNKS):
            step(c)
    for c in range(N_CHUNKS):
        final_step(c, last=(c == N_CHUNKS - 1))


def build_nc():
    nc = bacc.Bacc("TRN2", target_bir_lowering=False, debug=False)
    XT = nc.dram_tensor("XT", [N_IN, B_CORE], BF16, kind="ExternalInput")
    WeT = nc.dram_tensor("WeT", [N_IN, M], BF16, kind="ExternalInput")
    ST = nc.dram_tensor("ST", [M, M], BF16, kind="ExternalInput")
    THP = nc.dram_tensor("THP", [128, 128], FP32, kind="ExternalInput")
    ZT = nc.dram_tensor("ZT", [M, B_CORE], FP32, kind="ExternalOutput")
    with tile.TileContext(nc) as tc:
        with ExitStack() as ctx:
            _emit(
                ctx, tc, XT.ap(), WeT.ap(), ST.ap(), THP.ap(), ZT.ap(),
            )
    nc.compile()
    return nc


_NC_CACHE = None


def _get_nc():
    global _NC_CACHE
    if _NC_CACHE is None:
        _NC_CACHE = build_nc()
    return _NC_CACHE


def make_in_maps(X, We, S, theta):
    X = np.asarray(X, dtype=np.float32)
    WeT = np.ascontiguousarray(np.asarray(We, dtype=np.float32).T).astype(
        ml_dtypes.bfloat16
    )
    ST = np.ascontiguousarray(np.asarray(S, dtype=np.float32).T).astype(
        ml_dtypes.bfloat16
    )
    theta = np.asarray(theta, dtype=np.float32)
    # Padded [128, 128] per-partition layout: col kt holds theta[kt*128+p],
    # col KT+kt holds -theta[kt*128+p]; remaining columns are zero.
    thp = np.zeros((128, 128), dtype=np.float32)
    thp[:, :KT] = theta.reshape(KT, 128).T
    thp[:, KT : 2 * KT] = -thp[:, :KT]
    return [
        {
            "XT": np.ascontiguousarray(X[i * B_CORE : (i + 1) * B_CORE].T).astype(
                ml_dtypes.bfloat16
            ),
            "WeT": WeT,
            "ST": ST,
            "THP": thp,
        }
        for i in range(N_CORES)
    ]


def gather_out(results):
    return np.concatenate(
        [
            np.asarray(results[i]["ZT"], dtype=np.float32).T
            for i in range(N_CORES)
        ],
        axis=0,
    )


def run(X, We, S, theta, trace=False, **trace_kwargs):
    nc = _get_nc()
    in_maps = make_in_maps(X, We, S, theta)
    # The PJRT compile callback can fail transiently ("CallFunctionObjArgs");
    # a retry in the same process succeeds.
    last_err = None
    for _attempt in range(3):
        try:
            res = bass_utils.run_bass_kernel_spmd(
                nc, in_maps, list(range(N_CORES)), trace=trace, **trace_kwargs
            )
            break
        except Exception as e:  # noqa: BLE001
            last_err = e
            time.sleep(2.0)
    else:
        raise last_err
    Z = gather_out(res.results)
    return Z.astype(np.float32, copy=False), res


def kernel(X, We, S, theta):
    Z, _ = run(X, We, S, theta, trace=False)
    return Z


# revision 49
# speedup vs baseline: 1.0015x; 1.0001x over previous
"""Trainium2 Bass kernel for LISTA (nn_LISTA_37976100831401).

Data-parallel sharding: batch 16384 -> 8 NeuronCores x 2048 rows.
We / S / theta are replicated on every core; no cross-device communication.

Per-core algorithm:
  B  = X @ We.T                 (2048, 1024)
  Z0 = soft(B);  Z_{t+1} = soft(B + Z_t @ S.T), t = 0..15
  soft(x) = relu(x - theta) - relu(-x - theta)

All matmul operands are bf16 (PSUM accumulation stays fp32).  On TRN2 the
measured back-to-back N=512 matmul period is 216 ns for bf16 vs 227 ns for
fp32r (the fp32r weight path costs ~13 ns/matmul extra), so bf16 runs the
2048x1024x1024 step matmuls at the PE streaming roofline.  Numerically the
bf16 pipeline lands at ~7e-3 relative error (gate is 2e-2): quantization
noise (~0.4%/step) does not compound destructively through the 16
soft-threshold steps.

Everything stays in the [feature, batch] orientation for all 16 steps:
  C.T = S @ Z.T + B.T  via  psum[j,b] += ST[k][:, j128].T @ ZT[k][:, b512]
so the matmul OUTPUT layout equals the INPUT layout of the next step.  The
device writes Z.T ([1024, 2048] per core) and the host transposes while
gathering -- this removes the baseline's flipped final step that re-derived
a batch-major layout by re-accumulating X@We.T on the PE.

The four 512-column batch chunks advance round-robin (c0..c3 per step), so
while chunk c's PSUM groups drain through DVE/ACT the PE streams the other
three chunks' matmuls (~41 us of cover for a ~3 us chain).

Opening: throwaway bf16 matmuls on memset tiles start as soon as the PE
engine boots, warming the HAM clock gate (1.2 -> 2.4 GHz takes ~3.4 us of
sustained PE activity) while the input DMAs run across all three DMA rings
(sync / scalar / gpsimd-triggered), so the b-phase and the first step round
start with their operands resident and the clock warm.
"""

import time
from contextlib import ExitStack

import numpy as np
import ml_dtypes

import concourse.bacc as bacc
import concourse.mybir as mybir
import concourse.tile as tile
from concourse import bass_utils

FP32 = mybir.dt.float32
BF16 = mybir.dt.bfloat16
AL = mybir.AluOpType
AF = mybir.ActivationFunctionType

N_CORES = 8
B_TOTAL, N_IN, M = 16384, 256, 1024
B_CORE = B_TOTAL // N_CORES  # 2048
T_STEPS = 16                 # scan length in the reference
CHUNK = 512                  # batch columns per PSUM group (= bank / max N)
N_CHUNKS = B_CORE // CHUNK   # 4
KT = M // 128                # 8 feature tiles of 128
NT = N_IN // 128             # 2 input-feature tiles
N_WARM = 12                  # HAM warmup matmuls
WARM_N = 512                 # moving width of warmup matmuls


def _emit(ctx: ExitStack, tc: tile.TileContext, XT, WeT, ST, THP, ZT):
    nc = tc.nc

    const_pool = ctx.enter_context(tc.tile_pool(name="const", bufs=1))
    psum_pool = ctx.enter_context(tc.tile_pool(name="psum", bufs=1, space="PSUM"))
    xt_pool = ctx.enter_context(tc.tile_pool(name="xt", bufs=1))
    bt_pool = ctx.enter_context(tc.tile_pool(name="bt", bufs=1))
    zt_pool = ctx.enter_context(tc.tile_pool(name="zt", bufs=1))
    tmp_pool = ctx.enter_context(tc.tile_pool(name="tmp", bufs=1))
    out_pool = ctx.enter_context(tc.tile_pool(name="zout", bufs=1))

    # ---- constants -------------------------------------------------------
    warm_a = const_pool.tile([128, 128], BF16, name="warm_a")
    warm_m = const_pool.tile([128, WARM_N], BF16, name="warm_m")
    # theta ships host-marshaled in one padded [128, 128] fp32 tile (512B
    # DMA lines; a bare [128, 8] layout would be 32B lines = 128 tiny
    # descriptors that stall the whole ring): cols 0..KT-1 hold
    # th[p, kt] = theta[kt*128+p], cols KT..2KT-1 hold -theta.
    thpad = const_pool.tile([128, 128], FP32, name="thpad")

    def th_col(jt):
        return thpad[:, jt : jt + 1]

    def nth_col(jt):
        return thpad[:, KT + jt : KT + jt + 1]
    wet = [const_pool.tile([128, M], BF16, name=f"wet{nt}") for nt in range(NT)]
    st = [const_pool.tile([128, M], BF16, name=f"st{kt}") for kt in range(KT)]

    def emit_warmup():
        # PE work with zero DMA dependencies: warms the HAM clock gate while
        # inputs stream in.  Results are never read.
        nc.vector.memset(warm_a[:], 1.0)
        nc.vector.memset(warm_m[:], 0.5)
        for i in range(N_WARM):
            pw = psum_pool.tile([128, WARM_N], FP32, name="pw", tag="tp", bufs=2)
            nc.tensor.matmul(pw[:], warm_a[:], warm_m[:], start=True, stop=True)

    def emit_dma_in():
        # Ring speeds vary run-to-run (the gpsimd-triggered ring is the
        # slowest), so the first b-group's four operand tiles lead three
        # different rings; S.T streams behind (needed only by the first
        # step round, ~25us in), with the slow gpsimd ring carrying the
        # mid-order tiles.
        nc.gpsimd.dma_start(thpad[:], THP)
        nc.scalar.dma_start(wet[0][:], WeT[0:128, :])
        nc.sync.dma_start(wet[1][:], WeT[128:256, :])
        xts[0] = [
            xt_pool.tile([128, CHUNK], BF16, name=f"xt{nt}", tag=f"xt{nt}", bufs=4)
            for nt in range(NT)
        ]
        nc.sync.dma_start(xts[0][0][:], XT[0:128, 0:CHUNK])
        nc.gpsimd.dma_start(xts[0][1][:], XT[128:256, 0:CHUNK])
        for c in range(1, N_CHUNKS):
            x_phase(c, nc.sync)
        st_eng = {0: nc.scalar, 1: nc.scalar, 2: nc.scalar, 3: nc.sync,
                  4: nc.gpsimd, 5: nc.gpsimd, 6: nc.gpsimd, 7: nc.sync}
        for kt in range(KT):
            st_eng[kt].dma_start(st[kt][:], ST[kt * 128 : (kt + 1) * 128, :])

    xts = {}  # chunk -> [NT] tiles [128, CHUNK]

    def x_phase(c, dma_eng):
        xts[c] = [
            xt_pool.tile([128, CHUNK], BF16, name=f"xt{nt}", tag=f"xt{nt}", bufs=4)
            for nt in range(NT)
        ]
        for nt in range(NT):
            dma_eng.dma_start(
                xts[c][nt][:],
                XT[nt * 128 : (nt + 1) * 128, c * CHUNK : (c + 1) * CHUNK],
            )

    # ---- per-chunk state -------------------------------------------------
    bts = {}  # chunk -> [KT] tiles [128, CHUNK] bf16  (B.T slab)
    zts = {}  # chunk -> [KT] tiles [128, CHUNK] bf16  (current Z.T)

    def b_group(c, jt):
        # One tile of B.T = We @ X.T ; Z0 = soft(B).  Each b-group is only
        # ~432 ns of PE work (K=256), so PSUM bank turnaround gates the PE:
        # the bank is read exactly once by two parallel half-copies
        # (ACT + DVE, ~450 ns release), and Z0 derives from the bf16 B tile:
        #   soft(B) = relu(B - th) + min(B + th, 0)
        # Alternate between the two PSUM rings so b-groups see all 8 banks.
        tag, nbufs = ("mm", 6) if jt % 2 == 0 else ("tp", 2)
        ps = psum_pool.tile([128, CHUNK], FP32, name="psb", tag=tag, bufs=nbufs)
        for nt in range(NT):
            nc.tensor.matmul(
                ps[:],
                wet[nt][:, jt * 128 : (jt + 1) * 128],
                xts[c][nt][:],
                start=(nt == 0),
                stop=(nt == NT - 1),
            )
        btile = bt_pool.tile(
            [128, CHUNK], BF16, name="btile", tag=f"bt{jt}", bufs=4
        )
        half = CHUNK // 2
        nc.scalar.copy(btile[:, :half], ps[:, :half])
        nc.vector.tensor_copy(btile[:, half:], ps[:, half:])
        af = tmp_pool.tile([128, CHUNK], BF16, name="afb", tag="af", bufs=3)
        nc.scalar.activation(
            af[:], btile[:], AF.Relu, bias=nth_col(jt), scale=1.0
        )
        df = tmp_pool.tile([128, CHUNK], BF16, name="dfb", tag="df", bufs=3)
        nc.vector.tensor_scalar(
            df[:], btile[:], th_col(jt), 0.0, op0=AL.add, op1=AL.min
        )
        z0 = zt_pool.tile([128, CHUNK], BF16, name="z0", tag=f"zt{jt}", bufs=5)
        nc.vector.tensor_add(z0[:], af[:], df[:])
        bts[c].append(btile)
        zts[c].append(z0)

    def b_phase(c):
        bts[c] = []
        zts[c] = []
        for jt in range(KT):
            b_group(c, jt)

    def step_group(c, jt, zcur, znew):
        # One tile of Z <- soft(B + Z @ S.T), in the [j, b] orientation.
        ps = psum_pool.tile([128, CHUNK], FP32, name="pss", tag="mm", bufs=6)
        for kt in range(KT):
            nc.tensor.matmul(
                ps[:],
                st[kt][:, jt * 128 : (jt + 1) * 128],
                zcur[kt][:],
                start=(kt == 0),
                stop=(kt == KT - 1),
            )
        ct = tmp_pool.tile([128, CHUNK], BF16, name="ct", tag="ct", bufs=3)
        nc.vector.tensor_add(ct[:], ps[:], bts[c][jt][:])
        af = tmp_pool.tile([128, CHUNK], BF16, name="afs", tag="af", bufs=3)
        nc.scalar.activation(
            af[:], ct[:], AF.Relu, bias=nth_col(jt), scale=1.0
        )
        df = tmp_pool.tile([128, CHUNK], BF16, name="dfs", tag="df", bufs=3)
        nc.scalar.activation(
            df[:], ct[:], AF.Relu, bias=nth_col(jt), scale=-1.0
        )
        zn = zt_pool.tile([128, CHUNK], BF16, name="zn", tag=f"zt{jt}", bufs=5)
        nc.vector.tensor_sub(zn[:], af[:], df[:])
        znew.append(zn)

    def step(c, weave_b=None):
        # weave_b: chunk whose b-groups are interleaved 1:1 with this step's
        # groups, so each b-PSUM-bank gets ~2.2 us of PE cover to drain.
        zcur = zts[c]
        znew = []
        for jt in range(KT):
            step_group(c, jt, zcur, znew)
            if weave_b is not None:
                b_group(weave_b, jt)
        zts[c] = znew

    def final_step(c, last=False):
        # Last step keeps fp32 all the way to the output tile; Z.T DMAs out
        # row-contiguous (the host transposes while gathering).
        zcur = zts[c]
        for jt in range(KT):
            ps = psum_pool.tile([128, CHUNK], FP32, name="psf", tag="mm", bufs=6)
            for kt in range(KT):
                nc.tensor.matmul(
                    ps[:],
                    st[kt][:, jt * 128 : (jt + 1) * 128],
                    zcur[kt][:],
                    start=(kt == 0),
                    stop=(kt == KT - 1),
                )
            if last and jt == KT - 1:
                # Very last tile: halve the op width and run the relu (ACT)
                # against the min-branch (DVE) in parallel to shorten the
                # post-matmul critical chain that sets the kernel tail.
                #   soft(C) = relu(C - th) + min(C + th, 0)
                half = CHUNK // 2
                cf = tmp_pool.tile([128, CHUNK], FP32, name="cf", tag="cf", bufs=3)
                af = tmp_pool.tile([128, CHUNK], FP32, name="aff", tag="af", bufs=3)
                df = tmp_pool.tile([128, CHUNK], FP32, name="dff", tag="df", bufs=3)
                zo = out_pool.tile([128, CHUNK], FP32, name="zo", tag="zo", bufs=4)
                for h in (slice(0, half), slice(half, CHUNK)):
                    nc.vector.tensor_add(cf[:, h], ps[:, h], bts[c][jt][:, h])
                    nc.scalar.activation(
                        af[:, h], cf[:, h], AF.Relu,
                        bias=nth_col(jt), scale=1.0,
                    )
                    nc.vector.tensor_scalar(
                        df[:, h], cf[:, h], th_col(jt), 0.0,
                        op0=AL.add, op1=AL.min,
                    )
                    nc.vector.tensor_add(zo[:, h], af[:, h], df[:, h])
                    dma_eng = nc.sync if h.start == 0 else nc.scalar
                    dma_eng.dma_start(
                        ZT[jt * 128 : (jt + 1) * 128,
                           c * CHUNK + h.start : c * CHUNK + h.stop],
                        zo[:, h],
                    )
                continue
            cf = tmp_pool.tile([128, CHUNK], FP32, name="cf", tag="cf", bufs=3)
            nc.vector.tensor_add(cf[:], ps[:], bts[c][jt][:])
            af = tmp_pool.tile([128, CHUNK], FP32, name="aff", tag="af", bufs=3)
            nc.scalar.activation(
                af[:], cf[:], AF.Relu, bias=nth_col(jt), scale=1.0
            )
            df = tmp_pool.tile([128, CHUNK], FP32, name="dff", tag="df", bufs=3)
            nc.scalar.activation(
                df[:], cf[:], AF.Relu, bias=nth_col(jt), scale=-1.0
            )
            zo = out_pool.tile([128, CHUNK], FP32, name="zo", tag="zo", bufs=4)
            nc.vector.tensor_sub(zo[:], af[:], df[:])
            dma_eng = nc.sync if jt % 2 == 0 else nc.scalar
            dma_eng.dma_start(
                ZT[jt * 128 : (jt + 1) * 128, c * CHUNK : (c + 1) * CHUNK], zo[:]
            )

    # ---- emission schedule ----------------------------------------------
    emit_warmup()
    emit_dma_in()
    b_phase(0)
    b_phase(1)
    bts[2] = []
    zts[2] = []
    bts[3] = []
    zts[3] = []
    step(0, weave_b=2)
    step(1, weave_b=3)
    step(2)
    step(3)
    for _ in range(T_STEPS - 2):
        for c in range(N_CHUNKS):
            step(c)
    for c in range(N_CHUNKS):
        final_step(c, last=(c == N_CHUNKS - 1))


def build_nc():
    nc = bacc.Bacc("TRN2", target_bir_lowering=False, debug=False)
    XT = nc.dram_tensor("XT", [N_IN, B_CORE], BF16, kind="ExternalInput")
    WeT = nc.dram_tensor("WeT", [N_IN, M], BF16, kind="ExternalInput")
    ST = nc.dram_tensor("ST", [M, M], BF16, kind="ExternalInput")
    THP = nc.dram_tensor("THP", [128, 128], FP32, kind="ExternalInput")
    ZT = nc.dram_tensor("ZT", [M, B_CORE], FP32, kind="ExternalOutput")
    with tile.TileContext(nc) as tc:
        with ExitStack() as ctx:
            _emit(
                ctx, tc, XT.ap(), WeT.ap(), ST.ap(), THP.ap(), ZT.ap(),
            )
    nc.compile()
    return nc


_NC_CACHE = None


def _get_nc():
    global _NC_CACHE
    if _NC_CACHE is None:
        _NC_CACHE = build_nc()
    return _NC_CACHE


def make_in_maps(X, We, S, theta):
    X = np.asarray(X, dtype=np.float32)
    WeT = np.ascontiguousarray(np.asarray(We, dtype=np.float32).T).astype(
        ml_dtypes.bfloat16
    )
    ST = np.ascontiguousarray(np.asarray(S, dtype=np.float32).T).astype(
        ml_dtypes.bfloat16
    )
    theta = np.asarray(theta, dtype=np.float32)
    # Padded [128, 128] per-partition layout: col kt holds theta[kt*128+p],
    # col KT+kt holds -theta[kt*128+p]; remaining columns are zero.
    thp = np.zeros((128, 128), dtype=np.float32)
    thp[:, :KT] = theta.reshape(KT, 128).T
    thp[:, KT : 2 * KT] = -thp[:, :KT]
    return [
        {
            "XT": np.ascontiguousarray(X[i * B_CORE : (i + 1) * B_CORE].T).astype(
                ml_dtypes.bfloat16
            ),
            "WeT": WeT,
            "ST": ST,
            "THP": thp,
        }
        for i in range(N_CORES)
    ]


def gather_out(results):
    return np.concatenate(
        [
            np.asarray(results[i]["ZT"], dtype=np.float32).T
            for i in range(N_CORES)
        ],
        axis=0,
    )


def run(X, We, S, theta, trace=False, **trace_kwargs):
    nc = _get_nc()
    in_maps = make_in_maps(X, We, S, theta)
    # The PJRT compile callback can fail transiently ("CallFunctionObjArgs");
    # a retry in the same process succeeds.
    last_err = None
    for _attempt in range(3):
        try:
            res = bass_utils.run_bass_kernel_spmd(
                nc, in_maps, list(range(N_CORES)), trace=trace, **trace_kwargs
            )
            break
        except Exception as e:  # noqa: BLE001
            last_err = e
            time.sleep(2.0)
    else:
        raise last_err
    Z = gather_out(res.results)
    return Z.astype(np.float32, copy=False), res


def kernel(X, We, S, theta):
    Z, _ = run(X, We, S, theta, trace=False)
    return Z


# revision 50
# speedup vs baseline: 1.0019x; 1.0004x over previous
"""Trainium2 Bass kernel for LISTA (nn_LISTA_37976100831401).

Data-parallel sharding: batch 16384 -> 8 NeuronCores x 2048 rows.
We / S / theta are replicated on every core; no cross-device communication.

Per-core algorithm:
  B  = X @ We.T                 (2048, 1024)
  Z0 = soft(B);  Z_{t+1} = soft(B + Z_t @ S.T), t = 0..15
  soft(x) = relu(x - theta) - relu(-x - theta)

All matmul operands are bf16 (PSUM accumulation stays fp32).  On TRN2 the
measured back-to-back N=512 matmul period is 216 ns for bf16 vs 227 ns for
fp32r (the fp32r weight path costs ~13 ns/matmul extra), so bf16 runs the
2048x1024x1024 step matmuls at the PE streaming roofline.  Numerically the
bf16 pipeline lands at ~7e-3 relative error (gate is 2e-2): quantization
noise (~0.4%/step) does not compound destructively through the 16
soft-threshold steps.

Everything stays in the [feature, batch] orientation for all 16 steps:
  C.T = S @ Z.T + B.T  via  psum[j,b] += ST[k][:, j128].T @ ZT[k][:, b512]
so the matmul OUTPUT layout equals the INPUT layout of the next step.  The
device writes Z.T ([1024, 2048] per core) and the host transposes while
gathering -- this removes the baseline's flipped final step that re-derived
a batch-major layout by re-accumulating X@We.T on the PE.

The four 512-column batch chunks advance round-robin (c0..c3 per step), so
while chunk c's PSUM groups drain through DVE/ACT the PE streams the other
three chunks' matmuls (~41 us of cover for a ~3 us chain).

Opening: 12 throwaway bf16 matmuls on memset tiles start as soon as the PE
engine boots, warming the HAM clock gate (1.2 -> 2.4 GHz takes ~3.4 us of
sustained PE activity) while the input DMAs run across all three DMA rings
(sync / scalar / gpsimd-triggered), so the b-phase starts with its operands
resident and the clock warm.  The b-phase b-groups for chunks 2/3 are woven
1:1 into step 1 of chunks 0/1: a b-group is only ~432 ns of PE work but its
PSUM bank needs ~2.2 us of ACT/DVE post-processing, so weaving gives each
bank a full step-group (~1.7 us) of PE cover.  theta ships host-marshaled
in a padded [128, 128] tile (a bare [128, 8] layout would DMA as 128 tiny
32B descriptors and stall its ring for ~8 us).
"""

import time
from contextlib import ExitStack

import numpy as np
import ml_dtypes

import concourse.bacc as bacc
import concourse.mybir as mybir
import concourse.tile as tile
from concourse import bass_utils

FP32 = mybir.dt.float32
BF16 = mybir.dt.bfloat16
AL = mybir.AluOpType
AF = mybir.ActivationFunctionType

N_CORES = 8
B_TOTAL, N_IN, M = 16384, 256, 1024
B_CORE = B_TOTAL // N_CORES  # 2048
T_STEPS = 16                 # scan length in the reference
CHUNK = 512                  # batch columns per PSUM group (= bank / max N)
N_CHUNKS = B_CORE // CHUNK   # 4
KT = M // 128                # 8 feature tiles of 128
NT = N_IN // 128             # 2 input-feature tiles
N_WARM = 12                  # HAM warmup matmuls
WARM_N = 512                 # moving width of warmup matmuls


def _emit(ctx: ExitStack, tc: tile.TileContext, XT, WeT, ST, THP, ZT):
    nc = tc.nc

    const_pool = ctx.enter_context(tc.tile_pool(name="const", bufs=1))
    psum_pool = ctx.enter_context(tc.tile_pool(name="psum", bufs=1, space="PSUM"))
    xt_pool = ctx.enter_context(tc.tile_pool(name="xt", bufs=1))
    bt_pool = ctx.enter_context(tc.tile_pool(name="bt", bufs=1))
    zt_pool = ctx.enter_context(tc.tile_pool(name="zt", bufs=1))
    tmp_pool = ctx.enter_context(tc.tile_pool(name="tmp", bufs=1))
    out_pool = ctx.enter_context(tc.tile_pool(name="zout", bufs=1))

    # ---- constants -------------------------------------------------------
    warm_a = const_pool.tile([128, 128], BF16, name="warm_a")
    warm_m = const_pool.tile([128, WARM_N], BF16, name="warm_m")
    # theta ships host-marshaled in one padded [128, 128] fp32 tile (512B
    # DMA lines; a bare [128, 8] layout would be 32B lines = 128 tiny
    # descriptors that stall the whole ring): cols 0..KT-1 hold
    # th[p, kt] = theta[kt*128+p], cols KT..2KT-1 hold -theta.
    thpad = const_pool.tile([128, 128], FP32, name="thpad")

    def th_col(jt):
        return thpad[:, jt : jt + 1]

    def nth_col(jt):
        return thpad[:, KT + jt : KT + jt + 1]
    wet = [const_pool.tile([128, M], BF16, name=f"wet{nt}") for nt in range(NT)]
    st = [const_pool.tile([128, M], BF16, name=f"st{kt}") for kt in range(KT)]

    def emit_warmup():
        # PE work with zero DMA dependencies: warms the HAM clock gate while
        # inputs stream in.  Results are never read.
        nc.vector.memset(warm_a[:], 1.0)
        nc.vector.memset(warm_m[:], 0.5)
        for i in range(N_WARM):
            pw = psum_pool.tile([128, WARM_N], FP32, name="pw", tag="tp", bufs=2)
            nc.tensor.matmul(pw[:], warm_a[:], warm_m[:], start=True, stop=True)

    def emit_dma_in():
        # Ring speeds vary run-to-run (the gpsimd-triggered ring is the
        # slowest), so the first b-group's four operand tiles lead three
        # different rings; S.T streams behind (needed only by the first
        # step round, ~25us in), with the slow gpsimd ring carrying the
        # mid-order tiles.
        nc.gpsimd.dma_start(thpad[:], THP)
        nc.scalar.dma_start(wet[0][:], WeT[0:128, :])
        nc.sync.dma_start(wet[1][:], WeT[128:256, :])
        xts[0] = [
            xt_pool.tile([128, CHUNK], BF16, name=f"xt{nt}", tag=f"xt{nt}", bufs=4)
            for nt in range(NT)
        ]
        nc.sync.dma_start(xts[0][0][:], XT[0:128, 0:CHUNK])
        nc.gpsimd.dma_start(xts[0][1][:], XT[128:256, 0:CHUNK])
        for c in range(1, N_CHUNKS):
            x_phase(c, nc.sync)
        st_eng = {0: nc.scalar, 1: nc.scalar, 2: nc.scalar, 3: nc.sync,
                  4: nc.gpsimd, 5: nc.gpsimd, 6: nc.gpsimd, 7: nc.sync}
        for kt in range(KT):
            st_eng[kt].dma_start(st[kt][:], ST[kt * 128 : (kt + 1) * 128, :])

    xts = {}  # chunk -> [NT] tiles [128, CHUNK]

    def x_phase(c, dma_eng):
        xts[c] = [
            xt_pool.tile([128, CHUNK], BF16, name=f"xt{nt}", tag=f"xt{nt}", bufs=4)
            for nt in range(NT)
        ]
        for nt in range(NT):
            dma_eng.dma_start(
                xts[c][nt][:],
                XT[nt * 128 : (nt + 1) * 128, c * CHUNK : (c + 1) * CHUNK],
            )

    # ---- per-chunk state -------------------------------------------------
    bts = {}  # chunk -> [KT] tiles [128, CHUNK] bf16  (B.T slab)
    zts = {}  # chunk -> [KT] tiles [128, CHUNK] bf16  (current Z.T)

    def b_group(c, jt):
        # One tile of B.T = We @ X.T ; Z0 = soft(B).  Each b-group is only
        # ~432 ns of PE work (K=256), so PSUM bank turnaround gates the PE:
        # the bank is read exactly once by two parallel half-copies
        # (ACT + DVE, ~450 ns release), and Z0 derives from the bf16 B tile:
        #   soft(B) = relu(B - th) + min(B + th, 0)
        # Alternate between the two PSUM rings so b-groups see all 8 banks.
        tag, nbufs = ("mm", 6) if jt % 2 == 0 else ("tp", 2)
        ps = psum_pool.tile([128, CHUNK], FP32, name="psb", tag=tag, bufs=nbufs)
        for nt in range(NT):
            nc.tensor.matmul(
                ps[:],
                wet[nt][:, jt * 128 : (jt + 1) * 128],
                xts[c][nt][:],
                start=(nt == 0),
                stop=(nt == NT - 1),
            )
        btile = bt_pool.tile(
            [128, CHUNK], BF16, name="btile", tag=f"bt{jt}", bufs=4
        )
        half = CHUNK // 2
        nc.scalar.copy(btile[:, :half], ps[:, :half])
        nc.vector.tensor_copy(btile[:, half:], ps[:, half:])
        af = tmp_pool.tile([128, CHUNK], BF16, name="afb", tag="af", bufs=3)
        nc.scalar.activation(
            af[:], btile[:], AF.Relu, bias=nth_col(jt), scale=1.0
        )
        df = tmp_pool.tile([128, CHUNK], BF16, name="dfb", tag="df", bufs=3)
        nc.vector.tensor_scalar(
            df[:], btile[:], th_col(jt), 0.0, op0=AL.add, op1=AL.min
        )
        z0 = zt_pool.tile([128, CHUNK], BF16, name="z0", tag=f"zt{jt}", bufs=5)
        nc.vector.tensor_add(z0[:], af[:], df[:])
        bts[c].append(btile)
        zts[c].append(z0)

    def b_phase(c):
        bts[c] = []
        zts[c] = []
        for jt in range(KT):
            b_group(c, jt)

    def step_group(c, jt, zcur, znew):
        # One tile of Z <- soft(B + Z @ S.T), in the [j, b] orientation.
        ps = psum_pool.tile([128, CHUNK], FP32, name="pss", tag="mm", bufs=6)
        for kt in range(KT):
            nc.tensor.matmul(
                ps[:],
                st[kt][:, jt * 128 : (jt + 1) * 128],
                zcur[kt][:],
                start=(kt == 0),
                stop=(kt == KT - 1),
            )
        ct = tmp_pool.tile([128, CHUNK], BF16, name="ct", tag="ct", bufs=3)
        nc.vector.tensor_add(ct[:], ps[:], bts[c][jt][:])
        af = tmp_pool.tile([128, CHUNK], BF16, name="afs", tag="af", bufs=3)
        nc.scalar.activation(
            af[:], ct[:], AF.Relu, bias=nth_col(jt), scale=1.0
        )
        df = tmp_pool.tile([128, CHUNK], BF16, name="dfs", tag="df", bufs=3)
        nc.scalar.activation(
            df[:], ct[:], AF.Relu, bias=nth_col(jt), scale=-1.0
        )
        zn = zt_pool.tile([128, CHUNK], BF16, name="zn", tag=f"zt{jt}", bufs=5)
        nc.vector.tensor_sub(zn[:], af[:], df[:])
        znew.append(zn)

    def step(c, weave_b=None):
        # weave_b: chunk whose b-groups are interleaved 1:1 with this step's
        # groups, so each b-PSUM-bank gets ~2.2 us of PE cover to drain.
        zcur = zts[c]
        znew = []
        for jt in range(KT):
            step_group(c, jt, zcur, znew)
            if weave_b is not None:
                b_group(weave_b, jt)
        zts[c] = znew

    def final_step(c, last=False):
        # Last step keeps fp32 all the way to the output tile; Z.T DMAs out
        # row-contiguous (the host transposes while gathering).
        zcur = zts[c]
        for jt in range(KT):
            ps = psum_pool.tile([128, CHUNK], FP32, name="psf", tag="mm", bufs=6)
            for kt in range(KT):
                nc.tensor.matmul(
                    ps[:],
                    st[kt][:, jt * 128 : (jt + 1) * 128],
                    zcur[kt][:],
                    start=(kt == 0),
                    stop=(kt == KT - 1),
                )
            if last and jt == KT - 1:
                # Very last tile: halve the op width and run the relu (ACT)
                # against the min-branch (DVE) in parallel to shorten the
                # post-matmul critical chain that sets the kernel tail.
                #   soft(C) = relu(C - th) + min(C + th, 0)
                half = CHUNK // 2
                cf = tmp_pool.tile([128, CHUNK], FP32, name="cf", tag="cf", bufs=3)
                af = tmp_pool.tile([128, CHUNK], FP32, name="aff", tag="af", bufs=3)
                df = tmp_pool.tile([128, CHUNK], FP32, name="dff", tag="df", bufs=3)
                zo = out_pool.tile([128, CHUNK], FP32, name="zo", tag="zo", bufs=4)
                for h in (slice(0, half), slice(half, CHUNK)):
                    nc.vector.tensor_add(cf[:, h], ps[:, h], bts[c][jt][:, h])
                    nc.scalar.activation(
                        af[:, h], cf[:, h], AF.Relu,
                        bias=nth_col(jt), scale=1.0,
                    )
                    nc.vector.tensor_scalar(
                        df[:, h], cf[:, h], th_col(jt), 0.0,
                        op0=AL.add, op1=AL.min,
                    )
                    nc.vector.tensor_add(zo[:, h], af[:, h], df[:, h])
                    dma_eng = nc.sync if h.start == 0 else nc.scalar
                    dma_eng.dma_start(
                        ZT[jt * 128 : (jt + 1) * 128,
                           c * CHUNK + h.start : c * CHUNK + h.stop],
                        zo[:, h],
                    )
                continue
            cf = tmp_pool.tile([128, CHUNK], FP32, name="cf", tag="cf", bufs=3)
            nc.vector.tensor_add(cf[:], ps[:], bts[c][jt][:])
            af = tmp_pool.tile([128, CHUNK], FP32, name="aff", tag="af", bufs=3)
            nc.scalar.activation(
                af[:], cf[:], AF.Relu, bias=nth_col(jt), scale=1.0
            )
            df = tmp_pool.tile([128, CHUNK], FP32, name="dff", tag="df", bufs=3)
            nc.scalar.activation(
                df[:], cf[:], AF.Relu, bias=nth_col(jt), scale=-1.0
            )
            zo = out_pool.tile([128, CHUNK], FP32, name="zo", tag="zo", bufs=4)
            nc.vector.tensor_sub(zo[:], af[:], df[:])
            dma_eng = nc.sync if jt % 2 == 0 else nc.scalar
            dma_eng.dma_start(
                ZT[jt * 128 : (jt + 1) * 128, c * CHUNK : (c + 1) * CHUNK], zo[:]
            )

    # ---- emission schedule ----------------------------------------------
    emit_warmup()
    emit_dma_in()
    b_phase(0)
    b_phase(1)
    bts[2] = []
    zts[2] = []
    bts[3] = []
    zts[3] = []
    step(0, weave_b=2)
    step(1, weave_b=3)
    step(2)
    step(3)
    for _ in range(T_STEPS - 2):
        for c in range(N_CHUNKS):
            step(c)
    for c in range(N_CHUNKS):
        final_step(c, last=(c == N_CHUNKS - 1))


def build_nc():
    nc = bacc.Bacc("TRN2", target_bir_lowering=False, debug=False)
    XT = nc.dram_tensor("XT", [N_IN, B_CORE], BF16, kind="ExternalInput")
    WeT = nc.dram_tensor("WeT", [N_IN, M], BF16, kind="ExternalInput")
    ST = nc.dram_tensor("ST", [M, M], BF16, kind="ExternalInput")
    THP = nc.dram_tensor("THP", [128, 128], FP32, kind="ExternalInput")
    ZT = nc.dram_tensor("ZT", [M, B_CORE], FP32, kind="ExternalOutput")
    with tile.TileContext(nc) as tc:
        with ExitStack() as ctx:
            _emit(
                ctx, tc, XT.ap(), WeT.ap(), ST.ap(), THP.ap(), ZT.ap(),
            )
    nc.compile()
    return nc


_NC_CACHE = None


def _get_nc():
    global _NC_CACHE
    if _NC_CACHE is None:
        _NC_CACHE = build_nc()
    return _NC_CACHE


def make_in_maps(X, We, S, theta):
    X = np.asarray(X, dtype=np.float32)
    WeT = np.ascontiguousarray(np.asarray(We, dtype=np.float32).T).astype(
        ml_dtypes.bfloat16
    )
    ST = np.ascontiguousarray(np.asarray(S, dtype=np.float32).T).astype(
        ml_dtypes.bfloat16
    )
    theta = np.asarray(theta, dtype=np.float32)
    # Padded [128, 128] per-partition layout: col kt holds theta[kt*128+p],
    # col KT+kt holds -theta[kt*128+p]; remaining columns are zero.
    thp = np.zeros((128, 128), dtype=np.float32)
    thp[:, :KT] = theta.reshape(KT, 128).T
    thp[:, KT : 2 * KT] = -thp[:, :KT]
    return [
        {
            "XT": np.ascontiguousarray(X[i * B_CORE : (i + 1) * B_CORE].T).astype(
                ml_dtypes.bfloat16
            ),
            "WeT": WeT,
            "ST": ST,
            "THP": thp,
        }
        for i in range(N_CORES)
    ]


def gather_out(results):
    return np.concatenate(
        [
            np.asarray(results[i]["ZT"], dtype=np.float32).T
            for i in range(N_CORES)
        ],
        axis=0,
    )


def run(X, We, S, theta, trace=False, **trace_kwargs):
    nc = _get_nc()
    in_maps = make_in_maps(X, We, S, theta)
    # The PJRT compile callback can fail transiently ("CallFunctionObjArgs");
    # a retry in the same process succeeds.
    last_err = None
    for _attempt in range(3):
        try:
            res = bass_utils.run_bass_kernel_spmd(
                nc, in_maps, list(range(N_CORES)), trace=trace, **trace_kwargs
            )
            break
        except Exception as e:  # noqa: BLE001
            last_err = e
            time.sleep(2.0)
    else:
        raise last_err
    Z = gather_out(res.results)
    return Z.astype(np.float32, copy=False), res


def kernel(X, We, S, theta):
    Z, _ = run(X, We, S, theta, trace=False)
    return Z
